# revision 77
# baseline (speedup 1.0000x reference)
"""Trainium2 Bass kernel for nn_DelayedMLP (B=8, S=2048, I=1024, H=4096, O=1024).

Sharding: data-parallel over batch — core b computes batch row b.

All three matmuls run as fp8e4 with DoubleRow perf mode (two K=128 subtiles
contracted per instruction at 0.5 cycles/output column):
  gate  : direct fp8 (z error ~3.6% -> decay error ~1% abs, tolerable)
  mm1/2 : hi+lo split, 3 products per k-tile (Wh@Xh + Wl@Xh + Wh@Xl),
          packed as 3 DoubleRow instructions per adjacent k-tile pair
          -> 0.75 cycles per k-tile per column vs 1.0 for bf16.

Weights are pre-scaled by 32 on the host so their fp8 lo parts stay out of
the e4m3 subnormal range; the 1/32 descale is folded into the activation
scale (gate sigmoid, mm1 relu) or the output bias add (mm2).

Single fused chunk pipeline (first chunks small so the MLP starts early):
per chunk c emit gate(c) -> scan chain(c) -> mm1(c) -> mm2(c-1). mm1 uses
k-pair-outer accumulation over 4-ht psum blocks and mm2 k-pair-outer over
oc psum groups so the PE can consume W1/W2 k-slices as the DMA stream
delivers them instead of stalling on the full 8MB tensor. Multiple psum
accumulation groups share a 2KB bank; only the first matmul touching a
bank carries start=True (start zeroes the whole 2KB region, later groups
write through their still-pending bytes).

Per-core dataflow (feature-major on chip):
  z32[i,s]   = sum_j 32*Wgh[i,j] * xh[j,s]                  (PE, fp8 DoubleRow)
  decayT     = sigmoid(z32/32 + bg)                         (ACT)
  immT       = x * decayT                                   (DVE)
  delayedT   = x - immT                                     (DVE, in place)
  bufsT      = scan(decayT, delayedT)                       (DVE tensor_tensor_scan)
  comb       = immT + bufsT (in place); ch=fp8(comb) (Pool), cl=comb-ch (DVE)
  psum1      = 32*b1 seeded by ACT Copy, matmuls accumulate (start=False)
  tmp        = relu(psum1/32)  bf16                         (ACT)
  hh         = fp8(tmp) (Pool copy);  hl = tmp - hh (DVE)
  out[s,o]   = psum2/32 + b2                                (PE + DVE stt)

Sigmoids of chunk c+1 are interleaved one-per-block into mm1(c)'s relu
stream (after the corresponding gate half has run), keeping ACT bursts
inside the psum-rotation margin. x arrives as per-chunk-contiguous fp8
hi+lo blocks in one DMA per chunk; W1/W2 stream as k-pair slices matching
the kp-outer consumption order (hi tensors before lo).
"""

import numpy as np

import concourse.bass as bass
import concourse.mybir as mybir
import concourse.tile as tile
from concourse import bacc, bass_utils

P = 128
B, S, I, H, O = 8, 2048, 1024, 4096, 1024
KI = I // P           # 8 contraction subtiles over I
KH = H // P           # 32 contraction subtiles over H
CM = 256              # steady-state chunk size
OC = 512              # mm2 output free-dim chunk
SW = 32.0             # host-side weight scale (fp8 lo-part range)
ISW = 1.0 / SW

BF16 = mybir.dt.bfloat16
F32 = mybir.dt.float32
E4 = mybir.dt.float8e4
AF = mybir.ActivationFunctionType
ALU = mybir.AluOpType
DR = mybir.MatmulPerfMode.DoubleRow
NP_BF16 = mybir.dt.np(BF16)
NP_E4 = mybir.dt.np(E4)


def chunk_schedule(S_: int):
    # small leading chunks so mm1 starts early; steady state CM
    head, tail = [128, 128], [128, 128]
    rest = S_ - sum(head) - sum(tail)
    if rest < 0:
        head, tail = [128], [128]
        rest = S_ - 256
    assert rest % CM == 0
    return head + [CM] * (rest // CM) + tail


def build(nc: bass.Bass, S_: int = S):
    CS = chunk_schedule(S_)
    nch = len(CS)

    xC = nc.dram_tensor("xC", [P, 2 * KI * S_], E4, kind="ExternalInput").ap()
    wghT = nc.dram_tensor("WghT", [I, I], E4, kind="ExternalInput").ap()
    w1hT = nc.dram_tensor("W1hT", [I, H], E4, kind="ExternalInput").ap()
    w1lT = nc.dram_tensor("W1lT", [I, H], E4, kind="ExternalInput").ap()
    w2hT = nc.dram_tensor("W2hT", [H, O], E4, kind="ExternalInput").ap()
    w2lT = nc.dram_tensor("W2lT", [H, O], E4, kind="ExternalInput").ap()
    bgT = nc.dram_tensor("bgT", [P, KI], F32, kind="ExternalInput").ap()
    b1T = nc.dram_tensor("b1T", [P, KH], F32, kind="ExternalInput").ap()
    b2r = nc.dram_tensor("b2r", [1, O], E4, kind="ExternalInput").ap()
    out = nc.dram_tensor("out", [S_, O], BF16, kind="ExternalOutput").ap()

    vwg = wghT.rearrange("(ko p) i -> p ko i", p=P)
    vw1h = w1hT.rearrange("(ko p) h -> p ko h", p=P)
    vw1l = w1lT.rearrange("(ko p) h -> p ko h", p=P)
    vw2h = w2hT.rearrange("(kh p) o -> p kh o", p=P)
    vw2l = w2lT.rearrange("(kh p) o -> p kh o", p=P)

    with tile.TileContext(nc) as tc:
        with tc.tile_pool(name="const", bufs=1) as cp, \
             tc.tile_pool(name="comb", bufs=2) as combp, \
             tc.tile_pool(name="w1", bufs=1) as w1p, \
             tc.tile_pool(name="w2", bufs=1) as w2p, \
             tc.tile_pool(name="wg", bufs=1) as wgp, \
             tc.tile_pool(name="hid", bufs=2) as hidp, \
             tc.tile_pool(name="px", bufs=2) as px, \
             tc.tile_pool(name="p1a", bufs=2) as p1a, \
             tc.tile_pool(name="p1d", bufs=1) as p1d, \
             tc.tile_pool(name="p1s", bufs=1) as p1s, \
             tc.tile_pool(name="p2t", bufs=2) as p2t, \
             tc.tile_pool(name="outp", bufs=2) as outp, \
             tc.tile_pool(name="gps", bufs=2, space="PSUM") as gps, \
             tc.tile_pool(name="hps", bufs=2, space="PSUM") as hps, \
             tc.tile_pool(name="ops", bufs=1, space="PSUM") as ops:
            bg_sb = cp.tile([P, KI], F32, tag="bg")
            b1_sb = cp.tile([P, KH], F32, tag="b1")
            b2_sb = cp.tile([1, O], E4, tag="b2")
            b2full = cp.tile([P, O], E4, tag="b2full")

            wg_sb = wgp.tile([P, KI, I], E4, tag="wg", name="wg")
            w1h_sb = w1p.tile([P, KI, H], E4, tag="w1h", name="w1h")
            w1l_sb = w1p.tile([P, KI, H], E4, tag="w1l", name="w1l")
            w2h_sb = w2p.tile([P, KH, O], E4, tag="w2h", name="w2h")
            w2l_sb = w2p.tile([P, KH, O], E4, tag="w2l", name="w2l")
            ch = [combp.tile([P, KI, CM], E4, tag="ch", name=f"ch{i}")
                  for i in range(2)]
            cl = [combp.tile([P, KI, CM], E4, tag="cl", name=f"cl{i}")
                  for i in range(2)]
            hh = [hidp.tile([P, KH, CM], E4, tag="hh", name=f"hh{i}")
                  for i in range(2)]
            hl = [hidp.tile([P, KH, CM], E4, tag="hl", name=f"hl{i}")
                  for i in range(2)]


            def dma_x(c, split=False):
                o0 = sum(CS[:c])
                n = KI * CS[c]
                b0 = 2 * KI * o0
                xhl = px.tile([P, 2 * KI * CM], E4, tag="xhl", name=f"xhl{c}")
                nc.sync.dma_start(xhl[:, :n], xC[:, b0:b0 + n])
                if not split:
                    nc.sync.dma_start(xhl[:, n:2 * n],
                                      xC[:, b0 + n:b0 + 2 * n])
                return xhl

            def dma_x_lo(c, xhl):
                o0 = sum(CS[:c])
                n = KI * CS[c]
                b0 = 2 * KI * o0
                nc.sync.dma_start(xhl[:, n:2 * n], xC[:, b0 + n:b0 + 2 * n])

            def dma_w1(p0, p1):
                # one transfer per ko-pair; all hi pairs first, then lo,
                # matching mm1's two-pass (hi-only then lo) term order
                for t, v in ((w1h_sb, vw1h), (w1l_sb, vw1l)):
                    for kp in range(p0, p1):
                        ks = slice(2 * kp, 2 * kp + 2)
                        nc.sync.dma_start(t[:, ks, :], v[:, ks, :])

            def dma_w2(p0, p1):
                # one transfer per 2 kh-pairs; hi before lo
                for t, v in ((w2h_sb, vw2h), (w2l_sb, vw2l)):
                    for kp in range(p0, p1, 2):
                        ks = slice(2 * kp, 2 * min(kp + 2, p1))
                        nc.sync.dma_start(t[:, ks, :], v[:, ks, :])

            def gate_mm_half(c, xhl, half):
                C = CS[c]
                xh_sb = xhl[:, :KI * C]
                pss = [gps.tile([P, 2, CM], F32, tag="g",
                                name=f"g{c}_{half}_{t}") for t in range(2)]
                for kp in range(KI // 2):
                    xv = xh_sb[:, 2 * kp * C:(2 * kp + 2) * C].rearrange(
                        "p (k c) -> p k c", k=2)
                    for t in range(2):
                        for j in range(2):
                            it = 4 * half + 2 * t + j
                            nc.tensor.matmul(
                                pss[t][:, j, :C],
                                wg_sb[:, 2 * kp:2 * kp + 2,
                                      it * P:(it + 1) * P],
                                xv,
                                start=(kp == 0 and j == 0),
                                stop=(kp == KI // 2 - 1 and j == 1),
                                perf_mode=DR)
                return pss

            def sig_half(c, dec, pss, half):
                C = CS[c]
                for t in range(2):
                    for j in range(2):
                        it = 4 * half + 2 * t + j
                        nc.scalar.activation(dec[:, it, :C], pss[t][:, j, :C],
                                             AF.Sigmoid,
                                             bias=bg_sb[:, it:it + 1],
                                             scale=ISW)

            def sig_one(c, dec, pss, half, k):
                C = CS[c]
                it = 4 * half + k
                nc.scalar.activation(dec[:, it, :C], pss[k // 2][:, k % 2, :C],
                                     AF.Sigmoid, bias=bg_sb[:, it:it + 1],
                                     scale=ISW)

            def chain(c, dec, xhl, prev_binit):
                C = CS[c]
                xh_sb = xhl[:, :KI * C].rearrange("p (k c) -> p k c", k=KI)
                xl_sb = xhl[:, KI * C:2 * KI * C].rearrange(
                    "p (k c) -> p k c", k=KI)
                xb_sb = p1d.tile([P, KI, CM], BF16, tag="xb", name=f"xb{c}")
                nc.vector.tensor_add(xb_sb[:, :, :C], xh_sb, xl_sb)
                imm = p1d.tile([P, KI, CM], BF16, tag="imm", name=f"imm{c}")
                nc.vector.tensor_mul(imm[:, :, :C], dec[:, :, :C],
                                     xb_sb[:, :, :C])
                # delayed = x - imm, in place over the x tile
                nc.vector.tensor_sub(xb_sb[:, :, :C], xb_sb[:, :, :C],
                                     imm[:, :, :C])
                bf = p1s.tile([P, KI, CM], BF16, tag="bufs", name=f"bufs{c}")
                for it in range(KI):
                    init = 0.0 if prev_binit is None \
                        else prev_binit[:, it, 0:1]
                    nc.vector.tensor_tensor_scan(
                        bf[:, it, :C], dec[:, it, :C], xb_sb[:, it, :C], init,
                        op0=ALU.mult, op1=ALU.add)
                binit = p1a.tile([P, KI, 1], BF16, tag="binit",
                                 name=f"binit{c}")
                nc.vector.tensor_copy(binit[:], bf[:, :, C - 1:C])
                # comb = imm + bufs, in place over imm
                nc.vector.tensor_add(imm[:, :, :C], imm[:, :, :C],
                                     bf[:, :, :C])
                nc.gpsimd.tensor_copy(ch[c % 2][:, :, :C], imm[:, :, :C])
                nc.vector.tensor_sub(cl[c % 2][:, :, :C], imm[:, :, :C],
                                     ch[c % 2][:, :, :C])
                return binit

            def mm1_blk(c, blk):
                C = CS[c]
                chc, clc = ch[c % 2], cl[c % 2]
                hhc, hlc = hh[c % 2], hl[c % 2]
                npair = KI // 2
                ps = hps.tile([P, 4, CM], F32, tag="h", name=f"h{c}_{blk}")
                h4 = slice(blk * 4, blk * 4 + 4)
                # seed the psum with 32*b1 (broadcast along columns); the
                # matmuls then accumulate with start=False onto it
                nc.scalar.activation(
                    ps[:, :, :C],
                    b1_sb[:, h4, None].broadcast_to([P, 4, C]),
                    AF.Copy)
                for kp in range(npair):
                    ks = slice(2 * kp, 2 * kp + 2)
                    for j in range(4):
                        ht = blk * 4 + j
                        hs = slice(ht * P, (ht + 1) * P)
                        nc.tensor.matmul(
                            ps[:, j, :C], w1h_sb[:, ks, hs], chc[:, ks, :C],
                            start=False, stop=False, perf_mode=DR,
                            skip_group_check=True)
                for kp in range(npair):
                    ks = slice(2 * kp, 2 * kp + 2)
                    for j in range(4):
                        ht = blk * 4 + j
                        hs = slice(ht * P, (ht + 1) * P)
                        nc.tensor.matmul(
                            ps[:, j, :C], w1l_sb[:, ks, hs], chc[:, ks, :C],
                            start=False, stop=False, perf_mode=DR,
                            skip_group_check=True)
                        nc.tensor.matmul(
                            ps[:, j, :C], w1h_sb[:, ks, hs], clc[:, ks, :C],
                            start=False, stop=False,
                            perf_mode=DR, skip_group_check=True)
                tmp = p2t.tile([P, 4, CM], BF16, tag="tmp",
                               name=f"tmp{c}_{blk}")
                nc.scalar.activation(tmp[:, :, :C], ps[:, :, :C],
                                     AF.Relu, scale=ISW)
                nc.gpsimd.tensor_copy(hhc[:, h4, :C], tmp[:, :, :C])
                nc.vector.tensor_sub(hlc[:, h4, :C], tmp[:, :, :C],
                                     hhc[:, h4, :C])

            def mm2(c, tail=False):
                C = CS[c]
                r0 = sum(CS[:c])
                hhc, hlc = hh[c % 2], hl[c % 2]
                npair = KH // 2
                if tail:
                    # drain-friendly: per-oc sequential sweeps so the stt +
                    # out DMA of group N overlap group N+1's matmuls
                    for ss in range(C // P):
                        sx = slice(ss * P, (ss + 1) * P)
                        for oc in range(O // OC):
                            ocs = slice(oc * OC, (oc + 1) * OC)
                            ps = ops.tile([P, OC], F32, tag=f"o{oc}",
                                          name=f"ox{c}_{ss}_{oc}")
                            for kp in range(npair):
                                ks = slice(2 * kp, 2 * kp + 2)
                                nc.tensor.matmul(
                                    ps[:], hhc[:, ks, sx], w2h_sb[:, ks, ocs],
                                    start=(kp == 0), stop=False, perf_mode=DR)
                                nc.tensor.matmul(
                                    ps[:], hlc[:, ks, sx], w2h_sb[:, ks, ocs],
                                    start=False, stop=False, perf_mode=DR)
                                nc.tensor.matmul(
                                    ps[:], hhc[:, ks, sx], w2l_sb[:, ks, ocs],
                                    start=False, stop=(kp == npair - 1),
                                    perf_mode=DR)
                            ot = outp.tile([P, OC], BF16, tag="otx",
                                           name=f"otx{c}_{ss}_{oc}")
                            nc.vector.scalar_tensor_tensor(
                                ot[:], ps[:], ISW, b2full[:, ocs],
                                op0=ALU.mult, op1=ALU.add)
                            nc.sync.dma_start(
                                out[r0 + ss * P:r0 + (ss + 1) * P, ocs], ot[:])
                    return
                for ss in range(C // P):
                    sx = slice(ss * P, (ss + 1) * P)
                    pss = [ops.tile([P, OC], F32, tag=f"o{oc}",
                                    name=f"o{c}_{ss}_{oc}")
                           for oc in range(O // OC)]
                    for kp in range(npair):
                        ks = slice(2 * kp, 2 * kp + 2)
                        for oc in range(O // OC):
                            ps = pss[oc]
                            ocs = slice(oc * OC, (oc + 1) * OC)
                            nc.tensor.matmul(
                                ps[:], hhc[:, ks, sx], w2h_sb[:, ks, ocs],
                                start=(kp == 0), stop=False, perf_mode=DR)
                            nc.tensor.matmul(
                                ps[:], hlc[:, ks, sx], w2h_sb[:, ks, ocs],
                                start=False, stop=False, perf_mode=DR)
                    for kp in range(npair):
                        ks = slice(2 * kp, 2 * kp + 2)
                        for oc in range(O // OC):
                            ps = pss[oc]
                            ocs = slice(oc * OC, (oc + 1) * OC)
                            nc.tensor.matmul(
                                ps[:], hhc[:, ks, sx], w2l_sb[:, ks, ocs],
                                start=False, stop=(kp == npair - 1),
                                perf_mode=DR)
                    ot = outp.tile([P, O], BF16, tag="ot",
                                   name=f"ot{c}_{ss}")
                    for oc in range(O // OC):
                        ocs = slice(oc * OC, (oc + 1) * OC)
                        nc.vector.scalar_tensor_tensor(
                            ot[:, ocs], pss[oc][:], ISW, b2full[:, ocs],
                            op0=ALU.mult, op1=ALU.add)
                    nc.sync.dma_start(
                        out[r0 + ss * P:r0 + (ss + 1) * P, :], ot[:])

            # ---- fused pipeline ----
            nc.sync.dma_start(wg_sb[:, 0:4, :], vwg[:, 0:4, :])
            xs = {0: dma_x(0, split=True)}
            nc.sync.dma_start(wg_sb[:, 4:8, :], vwg[:, 4:8, :])
            xs[1] = dma_x(1, split=True)
            nc.sync.dma_start(bg_sb[:], bgT)
            nc.sync.dma_start(b2_sb[:], b2r)
            nc.gpsimd.partition_broadcast(b2full[:], b2_sb[:])

            decs = {0: p1d.tile([P, KI, CM], BF16, tag="dec", name="dec0")}
            g0 = gate_mm_half(0, xs[0], 0)
            sig_half(0, decs[0], g0, 0)
            g1 = gate_mm_half(0, xs[0], 1)
            sig_half(0, decs[0], g1, 1)
            nc.sync.dma_start(w1h_sb[:, 0:2, :], vw1h[:, 0:2, :])
            dma_x_lo(0, xs[0])
            dma_x_lo(1, xs[1])
            binit = chain(0, decs[0], xs[0], None)
            nc.sync.dma_start(b1_sb[:], b1T)
            for kp in range(1, 4):
                nc.sync.dma_start(w1h_sb[:, 2 * kp:2 * kp + 2, :],
                                  vw1h[:, 2 * kp:2 * kp + 2, :])
            for kp in range(4):
                nc.sync.dma_start(w1l_sb[:, 2 * kp:2 * kp + 2, :],
                                  vw1l[:, 2 * kp:2 * kp + 2, :])

            ghalves = {}
            for c in range(nch):
                nxt = c + 1 < nch
                early = c < 2   # ACT is idle during the DMA-paced start
                if nxt:
                    decs[c + 1] = p1d.tile([P, KI, CM], BF16, tag="dec",
                                           name=f"dec{c + 1}")
                    ghalves[0] = gate_mm_half(c + 1, xs[c + 1], 0)
                    if early:
                        sig_half(c + 1, decs[c + 1], ghalves[0], 0)
                        ghalves[1] = gate_mm_half(c + 1, xs[c + 1], 1)
                        sig_half(c + 1, decs[c + 1], ghalves[1], 1)
                for blk in range(KH // 4):
                    mm1_blk(c, blk)
                    if nxt and not early and blk < 4:
                        sig_one(c + 1, decs[c + 1], ghalves[0], 0, blk)
                    if nxt and not early and blk == 3:
                        # all half-0 sigmoids are emitted: the gps psum
                        # tiles may now be reallocated for half 1
                        ghalves[1] = gate_mm_half(c + 1, xs[c + 1], 1)
                    if nxt and not early and blk >= 4:
                        sig_one(c + 1, decs[c + 1], ghalves[1], 1, blk - 4)
                if c + 2 < nch:
                    xs[c + 2] = dma_x(c + 2)
                if c == 0:
                    dma_w2(0, 8)
                if c == 1:
                    dma_w2(8, 16)
                if nxt:
                    binit = chain(c + 1, decs.pop(c + 1), xs[c + 1], binit)
                    xs.pop(c)
                if c > 0:
                    mm2(c - 1)
            mm2(nch - 1, tail=True)
    return nc


def make_nc(S_: int = S) -> bass.Bass:
    nc = bacc.Bacc("TRN2", target_bir_lowering=False, debug=False,
                   enable_asserts=False, dynamic_dma_scratch_size=1024)
    build(nc, S_)
    nc.compile()
    return nc


def split8(a: np.ndarray):
    hi = a.astype(NP_E4)
    lo = (a - hi.astype(np.float32)).astype(NP_E4)
    return hi, lo


def prep_in_maps(inputs: dict) -> list[dict]:
    x = np.asarray(inputs["x"], np.float32)
    Wg = np.asarray(inputs["Wg"], np.float32)
    W1 = np.asarray(inputs["W1"], np.float32)
    W2 = np.asarray(inputs["W2"], np.float32)
    bg = np.asarray(inputs["bg"], np.float32)
    b1 = np.asarray(inputs["b1"], np.float32)
    b2 = np.asarray(inputs["b2"], np.float32)

    w1h, w1l = split8(np.ascontiguousarray(W1.T) * SW)   # [j, h]
    w2h, w2l = split8(np.ascontiguousarray(W2.T) * SW)   # [h, o]
    shared = {
        "WghT": (np.ascontiguousarray(Wg.T) * SW).astype(NP_E4),  # [j, i]
        "W1hT": w1h, "W1lT": w1l,
        "W2hT": w2h, "W2lT": w2l,
        "bgT": np.ascontiguousarray(bg.reshape(KI, P).T),  # [p, it]
        "b1T": np.ascontiguousarray((b1 * SW).reshape(KH, P).T),
        "b2r": b2.astype(NP_E4).reshape(1, O),
    }
    S_ = x.shape[1]
    CS = chunk_schedule(S_)
    in_maps = []
    for b in range(B):
        m = dict(shared)
        xT = np.ascontiguousarray(x[b].T)                  # [i, s]
        # per-chunk contiguous layout [P, KI*C per chunk] so each chunk is
        # one DMA with KI*C contiguous bytes per partition row
        blocks = []
        o0 = 0
        for C in CS:
            blk = xT[:, o0:o0 + C].reshape(KI, P, C).transpose(1, 0, 2)
            bh, bl = split8(blk.reshape(P, KI * C))
            blocks.append(bh)
            blocks.append(bl)
            o0 += C
        m["xC"] = np.ascontiguousarray(np.concatenate(blocks, axis=1))
        in_maps.append(m)
    return in_maps


LAST_RESULTS = None


def kernel(**inputs) -> np.ndarray:
    global LAST_RESULTS
    nc = make_nc()
    in_maps = prep_in_maps(inputs)
    res = bass_utils.run_bass_kernel_spmd(nc, in_maps, core_ids=list(range(B)))
    LAST_RESULTS = res
    out = np.stack([r["out"] for r in res.results], axis=0)
    return out.astype(np.float32)


# revision 79
# speedup vs baseline: 1.0020x; 1.0020x over previous
"""Trainium2 Bass kernel for nn_DelayedMLP (B=8, S=2048, I=1024, H=4096, O=1024).

Sharding: data-parallel over batch — core b computes batch row b.

All three matmuls run as fp8e4 with DoubleRow perf mode (two K=128 subtiles
contracted per instruction at 0.5 cycles/output column):
  gate  : direct fp8 (z error ~3.6% -> decay error ~1% abs, tolerable)
  mm1/2 : hi+lo split, 3 products per k-tile (Wh@Xh + Wl@Xh + Wh@Xl),
          packed as 3 DoubleRow instructions per adjacent k-tile pair
          -> 0.75 cycles per k-tile per column vs 1.0 for bf16.

Weights are pre-scaled by 32 on the host so their fp8 lo parts stay out of
the e4m3 subnormal range; the 1/32 descale is folded into the activation
scale (gate sigmoid, mm1 relu) or the output bias add (mm2).

Single fused chunk pipeline (first chunks small so the MLP starts early):
per chunk c emit gate(c) -> scan chain(c) -> mm1(c) -> mm2(c-1). mm1 uses
k-pair-outer accumulation over 4-ht psum blocks and mm2 k-pair-outer over
oc psum groups so the PE can consume W1/W2 k-slices as the DMA stream
delivers them instead of stalling on the full 8MB tensor. Multiple psum
accumulation groups share a 2KB bank; only the first matmul touching a
bank carries start=True (start zeroes the whole 2KB region, later groups
write through their still-pending bytes).

Per-core dataflow (feature-major on chip):
  z32[i,s]   = sum_j 32*Wgh[i,j] * xh[j,s]                  (PE, fp8 DoubleRow)
  decayT     = sigmoid(z32/32 + bg)                         (ACT)
  immT       = x * decayT                                   (DVE)
  delayedT   = x - immT                                     (DVE, in place)
  bufsT      = scan(decayT, delayedT)                       (DVE tensor_tensor_scan)
  comb       = immT + bufsT (in place); ch=fp8(comb) (Pool), cl=comb-ch (DVE)
  psum1      = 32*b1 seeded by ACT Copy, matmuls accumulate (start=False)
  tmp        = relu(psum1/32)  bf16                         (ACT)
  hh         = fp8(tmp) (Pool copy);  hl = tmp - hh (DVE)
  out[s,o]   = psum2/32 + b2                                (PE + DVE stt)

Sigmoids of chunk c+1 are interleaved one-per-block into mm1(c)'s relu
stream (after the corresponding gate half has run), keeping ACT bursts
inside the psum-rotation margin. x arrives as per-chunk-contiguous fp8
hi+lo blocks in one DMA per chunk; W1/W2 stream as k-pair slices matching
the kp-outer consumption order (hi tensors before lo).
"""

import numpy as np

import concourse.bass as bass
import concourse.mybir as mybir
import concourse.tile as tile
from concourse import bacc, bass_utils

P = 128
B, S, I, H, O = 8, 2048, 1024, 4096, 1024
KI = I // P           # 8 contraction subtiles over I
KH = H // P           # 32 contraction subtiles over H
CM = 256              # steady-state chunk size
OC = 512              # mm2 output free-dim chunk
SW = 32.0             # host-side weight scale (fp8 lo-part range)
ISW = 1.0 / SW

BF16 = mybir.dt.bfloat16
F32 = mybir.dt.float32
E4 = mybir.dt.float8e4
AF = mybir.ActivationFunctionType
ALU = mybir.AluOpType
DR = mybir.MatmulPerfMode.DoubleRow
NP_BF16 = mybir.dt.np(BF16)
NP_E4 = mybir.dt.np(E4)


def chunk_schedule(S_: int):
    # small leading chunks so mm1 starts early; steady state CM
    head, tail = [256], [256]
    rest = S_ - sum(head) - sum(tail)
    if rest < 0:
        head, tail = [128], [128]
        rest = S_ - 256
    assert rest % CM == 0
    return head + [CM] * (rest // CM) + tail


def build(nc: bass.Bass, S_: int = S):
    CS = chunk_schedule(S_)
    nch = len(CS)

    xC = nc.dram_tensor("xC", [P, 2 * KI * S_], E4, kind="ExternalInput").ap()
    wghT = nc.dram_tensor("WghT", [I, I], E4, kind="ExternalInput").ap()
    w1hT = nc.dram_tensor("W1hT", [I, H], E4, kind="ExternalInput").ap()
    w1lT = nc.dram_tensor("W1lT", [I, H], E4, kind="ExternalInput").ap()
    w2hT = nc.dram_tensor("W2hT", [H, O], E4, kind="ExternalInput").ap()
    w2lT = nc.dram_tensor("W2lT", [H, O], E4, kind="ExternalInput").ap()
    bgT = nc.dram_tensor("bgT", [P, KI], F32, kind="ExternalInput").ap()
    b1T = nc.dram_tensor("b1T", [P, KH], F32, kind="ExternalInput").ap()
    b2r = nc.dram_tensor("b2r", [1, O], E4, kind="ExternalInput").ap()
    out = nc.dram_tensor("out", [S_, O], BF16, kind="ExternalOutput").ap()

    vwg = wghT.rearrange("(ko p) i -> p ko i", p=P)
    vw1h = w1hT.rearrange("(ko p) h -> p ko h", p=P)
    vw1l = w1lT.rearrange("(ko p) h -> p ko h", p=P)
    vw2h = w2hT.rearrange("(kh p) o -> p kh o", p=P)
    vw2l = w2lT.rearrange("(kh p) o -> p kh o", p=P)

    with tile.TileContext(nc) as tc:
        with tc.tile_pool(name="const", bufs=1) as cp, \
             tc.tile_pool(name="comb", bufs=2) as combp, \
             tc.tile_pool(name="w1", bufs=1) as w1p, \
             tc.tile_pool(name="w2", bufs=1) as w2p, \
             tc.tile_pool(name="wg", bufs=1) as wgp, \
             tc.tile_pool(name="hid", bufs=2) as hidp, \
             tc.tile_pool(name="px", bufs=2) as px, \
             tc.tile_pool(name="p1a", bufs=2) as p1a, \
             tc.tile_pool(name="p1d", bufs=1) as p1d, \
             tc.tile_pool(name="p1s", bufs=1) as p1s, \
             tc.tile_pool(name="p2t", bufs=2) as p2t, \
             tc.tile_pool(name="outp", bufs=2) as outp, \
             tc.tile_pool(name="gps", bufs=2, space="PSUM") as gps, \
             tc.tile_pool(name="hps", bufs=2, space="PSUM") as hps, \
             tc.tile_pool(name="ops", bufs=1, space="PSUM") as ops:
            bg_sb = cp.tile([P, KI], F32, tag="bg")
            b1_sb = cp.tile([P, KH], F32, tag="b1")
            b2_sb = cp.tile([1, O], E4, tag="b2")
            b2full = cp.tile([P, O], E4, tag="b2full")

            wg_sb = wgp.tile([P, KI, I], E4, tag="wg", name="wg")
            w1h_sb = w1p.tile([P, KI, H], E4, tag="w1h", name="w1h")
            w1l_sb = w1p.tile([P, KI, H], E4, tag="w1l", name="w1l")
            w2h_sb = w2p.tile([P, KH, O], E4, tag="w2h", name="w2h")
            w2l_sb = w2p.tile([P, KH, O], E4, tag="w2l", name="w2l")
            ch = [combp.tile([P, KI, CM], E4, tag="ch", name=f"ch{i}")
                  for i in range(2)]
            cl = [combp.tile([P, KI, CM], E4, tag="cl", name=f"cl{i}")
                  for i in range(2)]
            hh = [hidp.tile([P, KH, CM], E4, tag="hh", name=f"hh{i}")
                  for i in range(2)]
            hl = [hidp.tile([P, KH, CM], E4, tag="hl", name=f"hl{i}")
                  for i in range(2)]


            def dma_x(c, split=False):
                o0 = sum(CS[:c])
                n = KI * CS[c]
                b0 = 2 * KI * o0
                xhl = px.tile([P, 2 * KI * CM], E4, tag="xhl", name=f"xhl{c}")
                nc.sync.dma_start(xhl[:, :n], xC[:, b0:b0 + n])
                if not split:
                    nc.sync.dma_start(xhl[:, n:2 * n],
                                      xC[:, b0 + n:b0 + 2 * n])
                return xhl

            def dma_x_lo(c, xhl):
                o0 = sum(CS[:c])
                n = KI * CS[c]
                b0 = 2 * KI * o0
                nc.sync.dma_start(xhl[:, n:2 * n], xC[:, b0 + n:b0 + 2 * n])

            def dma_w1(p0, p1):
                # one transfer per ko-pair; all hi pairs first, then lo,
                # matching mm1's two-pass (hi-only then lo) term order
                for t, v in ((w1h_sb, vw1h), (w1l_sb, vw1l)):
                    for kp in range(p0, p1):
                        ks = slice(2 * kp, 2 * kp + 2)
                        nc.sync.dma_start(t[:, ks, :], v[:, ks, :])

            def dma_w2(p0, p1):
                # one transfer per 2 kh-pairs; hi before lo
                for t, v in ((w2h_sb, vw2h), (w2l_sb, vw2l)):
                    for kp in range(p0, p1, 2):
                        ks = slice(2 * kp, 2 * min(kp + 2, p1))
                        nc.sync.dma_start(t[:, ks, :], v[:, ks, :])

            def gate_mm_half(c, xhl, half):
                C = CS[c]
                xh_sb = xhl[:, :KI * C]
                pss = [gps.tile([P, 2, CM], F32, tag="g",
                                name=f"g{c}_{half}_{t}") for t in range(2)]
                for kp in range(KI // 2):
                    xv = xh_sb[:, 2 * kp * C:(2 * kp + 2) * C].rearrange(
                        "p (k c) -> p k c", k=2)
                    for t in range(2):
                        for j in range(2):
                            it = 4 * half + 2 * t + j
                            nc.tensor.matmul(
                                pss[t][:, j, :C],
                                wg_sb[:, 2 * kp:2 * kp + 2,
                                      it * P:(it + 1) * P],
                                xv,
                                start=(kp == 0 and j == 0),
                                stop=(kp == KI // 2 - 1 and j == 1),
                                perf_mode=DR)
                return pss

            def sig_half(c, dec, pss, half):
                C = CS[c]
                for t in range(2):
                    for j in range(2):
                        it = 4 * half + 2 * t + j
                        nc.scalar.activation(dec[:, it, :C], pss[t][:, j, :C],
                                             AF.Sigmoid,
                                             bias=bg_sb[:, it:it + 1],
                                             scale=ISW)

            def sig_one(c, dec, pss, half, k):
                C = CS[c]
                it = 4 * half + k
                nc.scalar.activation(dec[:, it, :C], pss[k // 2][:, k % 2, :C],
                                     AF.Sigmoid, bias=bg_sb[:, it:it + 1],
                                     scale=ISW)

            def chain(c, dec, xhl, prev_binit):
                C = CS[c]
                xh_sb = xhl[:, :KI * C].rearrange("p (k c) -> p k c", k=KI)
                xl_sb = xhl[:, KI * C:2 * KI * C].rearrange(
                    "p (k c) -> p k c", k=KI)
                xb_sb = p1d.tile([P, KI, CM], BF16, tag="xb", name=f"xb{c}")
                nc.vector.tensor_add(xb_sb[:, :, :C], xh_sb, xl_sb)
                imm = p1d.tile([P, KI, CM], BF16, tag="imm", name=f"imm{c}")
                nc.vector.tensor_mul(imm[:, :, :C], dec[:, :, :C],
                                     xb_sb[:, :, :C])
                # delayed = x - imm, in place over the x tile
                nc.vector.tensor_sub(xb_sb[:, :, :C], xb_sb[:, :, :C],
                                     imm[:, :, :C])
                bf = p1s.tile([P, KI, CM], BF16, tag="bufs", name=f"bufs{c}")
                for it in range(KI):
                    init = 0.0 if prev_binit is None \
                        else prev_binit[:, it, 0:1]
                    nc.vector.tensor_tensor_scan(
                        bf[:, it, :C], dec[:, it, :C], xb_sb[:, it, :C], init,
                        op0=ALU.mult, op1=ALU.add)
                binit = p1a.tile([P, KI, 1], BF16, tag="binit",
                                 name=f"binit{c}")
                nc.vector.tensor_copy(binit[:], bf[:, :, C - 1:C])
                # comb = imm + bufs, in place over imm
                nc.vector.tensor_add(imm[:, :, :C], imm[:, :, :C],
                                     bf[:, :, :C])
                nc.gpsimd.tensor_copy(ch[c % 2][:, :, :C], imm[:, :, :C])
                nc.vector.tensor_sub(cl[c % 2][:, :, :C], imm[:, :, :C],
                                     ch[c % 2][:, :, :C])
                return binit

            def mm1_blk(c, blk):
                C = CS[c]
                chc, clc = ch[c % 2], cl[c % 2]
                hhc, hlc = hh[c % 2], hl[c % 2]
                npair = KI // 2
                ps = hps.tile([P, 4, CM], F32, tag="h", name=f"h{c}_{blk}")
                h4 = slice(blk * 4, blk * 4 + 4)
                # seed the psum with 32*b1 (broadcast along columns); the
                # matmuls then accumulate with start=False onto it
                nc.scalar.activation(
                    ps[:, :, :C],
                    b1_sb[:, h4, None].broadcast_to([P, 4, C]),
                    AF.Copy)
                for kp in range(npair):
                    ks = slice(2 * kp, 2 * kp + 2)
                    for j in range(4):
                        ht = blk * 4 + j
                        hs = slice(ht * P, (ht + 1) * P)
                        nc.tensor.matmul(
                            ps[:, j, :C], w1h_sb[:, ks, hs], chc[:, ks, :C],
                            start=False, stop=False, perf_mode=DR,
                            skip_group_check=True)
                for kp in range(npair):
                    ks = slice(2 * kp, 2 * kp + 2)
                    for j in range(4):
                        ht = blk * 4 + j
                        hs = slice(ht * P, (ht + 1) * P)
                        nc.tensor.matmul(
                            ps[:, j, :C], w1l_sb[:, ks, hs], chc[:, ks, :C],
                            start=False, stop=False, perf_mode=DR,
                            skip_group_check=True)
                        nc.tensor.matmul(
                            ps[:, j, :C], w1h_sb[:, ks, hs], clc[:, ks, :C],
                            start=False, stop=False,
                            perf_mode=DR, skip_group_check=True)
                tmp = p2t.tile([P, 4, CM], BF16, tag="tmp",
                               name=f"tmp{c}_{blk}")
                nc.scalar.activation(tmp[:, :, :C], ps[:, :, :C],
                                     AF.Relu, scale=ISW)
                nc.gpsimd.tensor_copy(hhc[:, h4, :C], tmp[:, :, :C])
                nc.vector.tensor_sub(hlc[:, h4, :C], tmp[:, :, :C],
                                     hhc[:, h4, :C])

            def mm2(c, tail=False):
                C = CS[c]
                r0 = sum(CS[:c])
                hhc, hlc = hh[c % 2], hl[c % 2]
                npair = KH // 2
                if tail:
                    # drain-friendly: per-oc sequential sweeps so the stt +
                    # out DMA of group N overlap group N+1's matmuls
                    for ss in range(C // P):
                        sx = slice(ss * P, (ss + 1) * P)
                        for oc in range(O // OC):
                            ocs = slice(oc * OC, (oc + 1) * OC)
                            ps = ops.tile([P, OC], F32, tag=f"o{oc}",
                                          name=f"ox{c}_{ss}_{oc}")
                            for kp in range(npair):
                                ks = slice(2 * kp, 2 * kp + 2)
                                nc.tensor.matmul(
                                    ps[:], hhc[:, ks, sx], w2h_sb[:, ks, ocs],
                                    start=(kp == 0), stop=False, perf_mode=DR)
                                nc.tensor.matmul(
                                    ps[:], hlc[:, ks, sx], w2h_sb[:, ks, ocs],
                                    start=False, stop=False, perf_mode=DR)
                                nc.tensor.matmul(
                                    ps[:], hhc[:, ks, sx], w2l_sb[:, ks, ocs],
                                    start=False, stop=(kp == npair - 1),
                                    perf_mode=DR)
                            ot = outp.tile([P, OC], BF16, tag="otx",
                                           name=f"otx{c}_{ss}_{oc}")
                            nc.vector.scalar_tensor_tensor(
                                ot[:], ps[:], ISW, b2full[:, ocs],
                                op0=ALU.mult, op1=ALU.add)
                            nc.sync.dma_start(
                                out[r0 + ss * P:r0 + (ss + 1) * P, ocs], ot[:])
                    return
                for ss in range(C // P):
                    sx = slice(ss * P, (ss + 1) * P)
                    pss = [ops.tile([P, OC], F32, tag=f"o{oc}",
                                    name=f"o{c}_{ss}_{oc}")
                           for oc in range(O // OC)]
                    for kp in range(npair):
                        ks = slice(2 * kp, 2 * kp + 2)
                        for oc in range(O // OC):
                            ps = pss[oc]
                            ocs = slice(oc * OC, (oc + 1) * OC)
                            nc.tensor.matmul(
                                ps[:], hhc[:, ks, sx], w2h_sb[:, ks, ocs],
                                start=(kp == 0), stop=False, perf_mode=DR)
                            nc.tensor.matmul(
                                ps[:], hlc[:, ks, sx], w2h_sb[:, ks, ocs],
                                start=False, stop=False, perf_mode=DR)
                    for kp in range(npair):
                        ks = slice(2 * kp, 2 * kp + 2)
                        for oc in range(O // OC):
                            ps = pss[oc]
                            ocs = slice(oc * OC, (oc + 1) * OC)
                            nc.tensor.matmul(
                                ps[:], hhc[:, ks, sx], w2l_sb[:, ks, ocs],
                                start=False, stop=(kp == npair - 1),
                                perf_mode=DR)
                    ot = outp.tile([P, O], BF16, tag="ot",
                                   name=f"ot{c}_{ss}")
                    for oc in range(O // OC):
                        ocs = slice(oc * OC, (oc + 1) * OC)
                        nc.vector.scalar_tensor_tensor(
                            ot[:, ocs], pss[oc][:], ISW, b2full[:, ocs],
                            op0=ALU.mult, op1=ALU.add)
                    nc.sync.dma_start(
                        out[r0 + ss * P:r0 + (ss + 1) * P, :], ot[:])

            # ---- fused pipeline ----
            nc.sync.dma_start(wg_sb[:, 0:4, :], vwg[:, 0:4, :])
            xs = {0: dma_x(0, split=True)}
            nc.sync.dma_start(wg_sb[:, 4:8, :], vwg[:, 4:8, :])
            xs[1] = dma_x(1, split=True)
            nc.sync.dma_start(bg_sb[:], bgT)
            nc.sync.dma_start(b2_sb[:], b2r)
            nc.gpsimd.partition_broadcast(b2full[:], b2_sb[:])

            decs = {0: p1d.tile([P, KI, CM], BF16, tag="dec", name="dec0")}
            g0 = gate_mm_half(0, xs[0], 0)
            sig_half(0, decs[0], g0, 0)
            g1 = gate_mm_half(0, xs[0], 1)
            sig_half(0, decs[0], g1, 1)
            nc.sync.dma_start(w1h_sb[:, 0:2, :], vw1h[:, 0:2, :])
            dma_x_lo(0, xs[0])
            dma_x_lo(1, xs[1])
            binit = chain(0, decs[0], xs[0], None)
            nc.sync.dma_start(b1_sb[:], b1T)
            for kp in range(1, 4):
                nc.sync.dma_start(w1h_sb[:, 2 * kp:2 * kp + 2, :],
                                  vw1h[:, 2 * kp:2 * kp + 2, :])
            for kp in range(4):
                nc.sync.dma_start(w1l_sb[:, 2 * kp:2 * kp + 2, :],
                                  vw1l[:, 2 * kp:2 * kp + 2, :])

            ghalves = {}
            for c in range(nch):
                nxt = c + 1 < nch
                early = c < 2   # ACT is idle during the DMA-paced start
                if nxt:
                    decs[c + 1] = p1d.tile([P, KI, CM], BF16, tag="dec",
                                           name=f"dec{c + 1}")
                    ghalves[0] = gate_mm_half(c + 1, xs[c + 1], 0)
                    if early:
                        sig_half(c + 1, decs[c + 1], ghalves[0], 0)
                        ghalves[1] = gate_mm_half(c + 1, xs[c + 1], 1)
                        sig_half(c + 1, decs[c + 1], ghalves[1], 1)
                for blk in range(KH // 4):
                    mm1_blk(c, blk)
                    if nxt and not early and blk < 4:
                        sig_one(c + 1, decs[c + 1], ghalves[0], 0, blk)
                    if nxt and not early and blk == 3:
                        # all half-0 sigmoids are emitted: the gps psum
                        # tiles may now be reallocated for half 1
                        ghalves[1] = gate_mm_half(c + 1, xs[c + 1], 1)
                    if nxt and not early and blk >= 4:
                        sig_one(c + 1, decs[c + 1], ghalves[1], 1, blk - 4)
                if c + 2 < nch:
                    xs[c + 2] = dma_x(c + 2)
                if c == 0:
                    dma_w2(0, 8)
                if c == 1:
                    dma_w2(8, 16)
                if nxt:
                    binit = chain(c + 1, decs.pop(c + 1), xs[c + 1], binit)
                    xs.pop(c)
                if c > 0:
                    mm2(c - 1)
            mm2(nch - 1, tail=True)
    return nc


def make_nc(S_: int = S) -> bass.Bass:
    nc = bacc.Bacc("TRN2", target_bir_lowering=False, debug=False,
                   enable_asserts=False, dynamic_dma_scratch_size=1024)
    build(nc, S_)
    nc.compile()
    return nc


def split8(a: np.ndarray):
    hi = a.astype(NP_E4)
    lo = (a - hi.astype(np.float32)).astype(NP_E4)
    return hi, lo


def prep_in_maps(inputs: dict) -> list[dict]:
    x = np.asarray(inputs["x"], np.float32)
    Wg = np.asarray(inputs["Wg"], np.float32)
    W1 = np.asarray(inputs["W1"], np.float32)
    W2 = np.asarray(inputs["W2"], np.float32)
    bg = np.asarray(inputs["bg"], np.float32)
    b1 = np.asarray(inputs["b1"], np.float32)
    b2 = np.asarray(inputs["b2"], np.float32)

    w1h, w1l = split8(np.ascontiguousarray(W1.T) * SW)   # [j, h]
    w2h, w2l = split8(np.ascontiguousarray(W2.T) * SW)   # [h, o]
    shared = {
        "WghT": (np.ascontiguousarray(Wg.T) * SW).astype(NP_E4),  # [j, i]
        "W1hT": w1h, "W1lT": w1l,
        "W2hT": w2h, "W2lT": w2l,
        "bgT": np.ascontiguousarray(bg.reshape(KI, P).T),  # [p, it]
        "b1T": np.ascontiguousarray((b1 * SW).reshape(KH, P).T),
        "b2r": b2.astype(NP_E4).reshape(1, O),
    }
    S_ = x.shape[1]
    CS = chunk_schedule(S_)
    in_maps = []
    for b in range(B):
        m = dict(shared)
        xT = np.ascontiguousarray(x[b].T)                  # [i, s]
        # per-chunk contiguous layout [P, KI*C per chunk] so each chunk is
        # one DMA with KI*C contiguous bytes per partition row
        blocks = []
        o0 = 0
        for C in CS:
            blk = xT[:, o0:o0 + C].reshape(KI, P, C).transpose(1, 0, 2)
            bh, bl = split8(blk.reshape(P, KI * C))
            blocks.append(bh)
            blocks.append(bl)
            o0 += C
        m["xC"] = np.ascontiguousarray(np.concatenate(blocks, axis=1))
        in_maps.append(m)
    return in_maps


LAST_RESULTS = None


def kernel(**inputs) -> np.ndarray:
    global LAST_RESULTS
    nc = make_nc()
    in_maps = prep_in_maps(inputs)
    res = bass_utils.run_bass_kernel_spmd(nc, in_maps, core_ids=list(range(B)))
    LAST_RESULTS = res
    out = np.stack([r["out"] for r in res.results], axis=0)
    return out.astype(np.float32)


# revision 84
# speedup vs baseline: 1.0052x; 1.0032x over previous
"""Trainium2 Bass kernel for nn_DelayedMLP (B=8, S=2048, I=1024, H=4096, O=1024).

Sharding: data-parallel over batch — core b computes batch row b.

All three matmuls run as fp8e4 with DoubleRow perf mode (two K=128 subtiles
contracted per instruction at 0.5 cycles/output column):
  gate  : direct fp8 (z error ~3.6% -> decay error ~1% abs, tolerable)
  mm1/2 : hi+lo split, 3 products per k-tile (Wh@Xh + Wl@Xh + Wh@Xl),
          packed as 3 DoubleRow instructions per adjacent k-tile pair
          -> 0.75 cycles per k-tile per column vs 1.0 for bf16.

Weights are pre-scaled by 32 on the host so their fp8 lo parts stay out of
the e4m3 subnormal range; the 1/32 descale is folded into the activation
scale (gate sigmoid, mm1 relu) or the output bias add (mm2).

Single fused chunk pipeline (first chunks small so the MLP starts early):
per chunk c emit gate(c) -> scan chain(c) -> mm1(c) -> mm2(c-1). mm1 uses
k-pair-outer accumulation over 4-ht psum blocks and mm2 k-pair-outer over
oc psum groups so the PE can consume W1/W2 k-slices as the DMA stream
delivers them instead of stalling on the full 8MB tensor. Multiple psum
accumulation groups share a 2KB bank; only the first matmul touching a
bank carries start=True (start zeroes the whole 2KB region, later groups
write through their still-pending bytes).

Per-core dataflow (feature-major on chip):
  z32[i,s]   = sum_j 32*Wgh[i,j] * xh[j,s]                  (PE, fp8 DoubleRow)
  decayT     = sigmoid(z32/32 + bg)                         (ACT)
  immT       = x * decayT                                   (DVE)
  delayedT   = x - immT                                     (DVE, in place)
  bufsT      = scan(decayT, delayedT)                       (DVE tensor_tensor_scan)
  comb       = immT + bufsT (in place); ch=fp8(comb) (Pool), cl=comb-ch (DVE)
  psum1      = 32*b1 seeded by ACT Copy, matmuls accumulate (start=False)
  tmp        = relu(psum1/32)  bf16                         (ACT)
  hh         = fp8(tmp) (Pool copy);  hl = tmp - hh (DVE)
  out[s,o]   = psum2/32 + b2                                (PE + DVE stt)

Sigmoids of chunk c+1 are interleaved one-per-block into mm1(c)'s relu
stream (after the corresponding gate half has run), keeping ACT bursts
inside the psum-rotation margin. x arrives as per-chunk-contiguous fp8
hi+lo blocks in one DMA per chunk; W1/W2 stream as k-pair slices matching
the kp-outer consumption order (hi tensors before lo).
"""

import numpy as np

import concourse.bass as bass
import concourse.mybir as mybir
import concourse.tile as tile
from concourse import bacc, bass_utils

P = 128
B, S, I, H, O = 8, 2048, 1024, 4096, 1024
KI = I // P           # 8 contraction subtiles over I
KH = H // P           # 32 contraction subtiles over H
CM = 256              # steady-state chunk size
OC = 512              # mm2 output free-dim chunk
SW = 32.0             # host-side weight scale (fp8 lo-part range)
ISW = 1.0 / SW

BF16 = mybir.dt.bfloat16
F32 = mybir.dt.float32
E4 = mybir.dt.float8e4
AF = mybir.ActivationFunctionType
ALU = mybir.AluOpType
DR = mybir.MatmulPerfMode.DoubleRow
NP_BF16 = mybir.dt.np(BF16)
NP_E4 = mybir.dt.np(E4)


def chunk_schedule(S_: int):
    # small leading chunks so mm1 starts early; steady state CM
    head, tail = [256], [256]
    rest = S_ - sum(head) - sum(tail)
    if rest < 0:
        head, tail = [128], [128]
        rest = S_ - 256
    assert rest % CM == 0
    return head + [CM] * (rest // CM) + tail


def build(nc: bass.Bass, S_: int = S):
    CS = chunk_schedule(S_)
    nch = len(CS)

    xC = nc.dram_tensor("xC", [P, 2 * KI * S_], E4, kind="ExternalInput").ap()
    wghT = nc.dram_tensor("WghT", [I, I], E4, kind="ExternalInput").ap()
    w1hT = nc.dram_tensor("W1hT", [I, H], E4, kind="ExternalInput").ap()
    w1lT = nc.dram_tensor("W1lT", [I, H], E4, kind="ExternalInput").ap()
    w2hT = nc.dram_tensor("W2hT", [H, O], E4, kind="ExternalInput").ap()
    w2lT = nc.dram_tensor("W2lT", [H, O], E4, kind="ExternalInput").ap()
    bgT = nc.dram_tensor("bgT", [P, KI], F32, kind="ExternalInput").ap()
    b1T = nc.dram_tensor("b1T", [P, KH], F32, kind="ExternalInput").ap()
    b2r = nc.dram_tensor("b2r", [1, O], E4, kind="ExternalInput").ap()
    out = nc.dram_tensor("out", [S_, O], BF16, kind="ExternalOutput").ap()

    vwg = wghT.rearrange("(ko p) i -> p ko i", p=P)
    vw1h = w1hT.rearrange("(ko p) h -> p ko h", p=P)
    vw1l = w1lT.rearrange("(ko p) h -> p ko h", p=P)
    vw2h = w2hT.rearrange("(kh p) o -> p kh o", p=P)
    vw2l = w2lT.rearrange("(kh p) o -> p kh o", p=P)

    with tile.TileContext(nc) as tc:
        with tc.tile_pool(name="const", bufs=1) as cp, \
             tc.tile_pool(name="comb", bufs=2) as combp, \
             tc.tile_pool(name="w1", bufs=1) as w1p, \
             tc.tile_pool(name="w2", bufs=1) as w2p, \
             tc.tile_pool(name="wg", bufs=1) as wgp, \
             tc.tile_pool(name="hid", bufs=2) as hidp, \
             tc.tile_pool(name="px", bufs=2) as px, \
             tc.tile_pool(name="p1a", bufs=2) as p1a, \
             tc.tile_pool(name="p1d", bufs=1) as p1d, \
             tc.tile_pool(name="p1s", bufs=1) as p1s, \
             tc.tile_pool(name="p2t", bufs=2) as p2t, \
             tc.tile_pool(name="outp", bufs=2) as outp, \
             tc.tile_pool(name="gps", bufs=2, space="PSUM") as gps, \
             tc.tile_pool(name="hps", bufs=2, space="PSUM") as hps, \
             tc.tile_pool(name="ops", bufs=1, space="PSUM") as ops:
            bg_sb = cp.tile([P, KI], F32, tag="bg")
            b1_sb = cp.tile([P, KH], F32, tag="b1")
            b2_sb = cp.tile([1, O], E4, tag="b2")
            b2full = cp.tile([P, O], E4, tag="b2full")

            wg_sb = wgp.tile([P, KI, I], E4, tag="wg", name="wg")
            w1h_sb = w1p.tile([P, KI, H], E4, tag="w1h", name="w1h")
            w1l_sb = w1p.tile([P, KI, H], E4, tag="w1l", name="w1l")
            w2h_sb = w2p.tile([P, KH, O], E4, tag="w2h", name="w2h")
            w2l_sb = w2p.tile([P, KH, O], E4, tag="w2l", name="w2l")
            ch = [combp.tile([P, KI, CM], E4, tag="ch", name=f"ch{i}")
                  for i in range(2)]
            cl = [combp.tile([P, KI, CM], E4, tag="cl", name=f"cl{i}")
                  for i in range(2)]
            hh = [hidp.tile([P, KH, CM], E4, tag="hh", name=f"hh{i}")
                  for i in range(2)]
            hl = [hidp.tile([P, KH, CM], E4, tag="hl", name=f"hl{i}")
                  for i in range(2)]


            def dma_x(c, split=False):
                o0 = sum(CS[:c])
                n = KI * CS[c]
                b0 = 2 * KI * o0
                xhl = px.tile([P, 2 * KI * CM], E4, tag="xhl", name=f"xhl{c}")
                nc.sync.dma_start(xhl[:, :n], xC[:, b0:b0 + n])
                if not split:
                    nc.sync.dma_start(xhl[:, n:2 * n],
                                      xC[:, b0 + n:b0 + 2 * n])
                return xhl

            def dma_x_lo(c, xhl):
                o0 = sum(CS[:c])
                n = KI * CS[c]
                b0 = 2 * KI * o0
                nc.sync.dma_start(xhl[:, n:2 * n], xC[:, b0 + n:b0 + 2 * n])

            def dma_w1(p0, p1):
                # one transfer per ko-pair; all hi pairs first, then lo,
                # matching mm1's two-pass (hi-only then lo) term order
                for t, v in ((w1h_sb, vw1h), (w1l_sb, vw1l)):
                    for kp in range(p0, p1):
                        ks = slice(2 * kp, 2 * kp + 2)
                        nc.sync.dma_start(t[:, ks, :], v[:, ks, :])

            def dma_w2(p0, p1):
                # one transfer per 2 kh-pairs; hi before lo
                for t, v in ((w2h_sb, vw2h), (w2l_sb, vw2l)):
                    for kp in range(p0, p1, 2):
                        ks = slice(2 * kp, 2 * min(kp + 2, p1))
                        nc.sync.dma_start(t[:, ks, :], v[:, ks, :])

            def gate_mm_half(c, xhl, half):
                C = CS[c]
                xh_sb = xhl[:, :KI * C]
                pss = [gps.tile([P, 2, CM], F32, tag="g",
                                name=f"g{c}_{half}_{t}") for t in range(2)]
                for kp in range(KI // 2):
                    xv = xh_sb[:, 2 * kp * C:(2 * kp + 2) * C].rearrange(
                        "p (k c) -> p k c", k=2)
                    for t in range(2):
                        for j in range(2):
                            it = 4 * half + 2 * t + j
                            nc.tensor.matmul(
                                pss[t][:, j, :C],
                                wg_sb[:, 2 * kp:2 * kp + 2,
                                      it * P:(it + 1) * P],
                                xv,
                                start=(kp == 0 and j == 0),
                                stop=(kp == KI // 2 - 1 and j == 1),
                                perf_mode=DR)
                return pss

            def sig_half(c, dec, pss, half):
                C = CS[c]
                for t in range(2):
                    for j in range(2):
                        it = 4 * half + 2 * t + j
                        nc.scalar.activation(dec[:, it, :C], pss[t][:, j, :C],
                                             AF.Sigmoid,
                                             bias=bg_sb[:, it:it + 1],
                                             scale=ISW)

            def sig_one(c, dec, pss, half, k):
                C = CS[c]
                it = 4 * half + k
                nc.scalar.activation(dec[:, it, :C], pss[k // 2][:, k % 2, :C],
                                     AF.Sigmoid, bias=bg_sb[:, it:it + 1],
                                     scale=ISW)

            def chain(c, dec, xhl, prev_binit):
                C = CS[c]
                xh_sb = xhl[:, :KI * C].rearrange("p (k c) -> p k c", k=KI)
                xl_sb = xhl[:, KI * C:2 * KI * C].rearrange(
                    "p (k c) -> p k c", k=KI)
                xb_sb = p1d.tile([P, KI, CM], BF16, tag="xb", name=f"xb{c}")
                nc.vector.tensor_add(xb_sb[:, :, :C], xh_sb, xl_sb)
                imm = p1d.tile([P, KI, CM], BF16, tag="imm", name=f"imm{c}")
                nc.vector.tensor_mul(imm[:, :, :C], dec[:, :, :C],
                                     xb_sb[:, :, :C])
                # delayed = x - imm, in place over the x tile
                nc.vector.tensor_sub(xb_sb[:, :, :C], xb_sb[:, :, :C],
                                     imm[:, :, :C])
                bf = p1s.tile([P, KI, CM], BF16, tag="bufs", name=f"bufs{c}")
                for it in range(KI):
                    init = 0.0 if prev_binit is None \
                        else prev_binit[:, it, 0:1]
                    nc.vector.tensor_tensor_scan(
                        bf[:, it, :C], dec[:, it, :C], xb_sb[:, it, :C], init,
                        op0=ALU.mult, op1=ALU.add)
                binit = p1a.tile([P, KI, 1], BF16, tag="binit",
                                 name=f"binit{c}")
                nc.vector.tensor_copy(binit[:], bf[:, :, C - 1:C])
                # comb = imm + bufs, in place over imm
                nc.vector.tensor_add(imm[:, :, :C], imm[:, :, :C],
                                     bf[:, :, :C])
                nc.gpsimd.tensor_copy(ch[c % 2][:, :, :C], imm[:, :, :C])
                nc.vector.tensor_sub(cl[c % 2][:, :, :C], imm[:, :, :C],
                                     ch[c % 2][:, :, :C])
                return binit

            def mm1_blk(c, blk):
                C = CS[c]
                chc, clc = ch[c % 2], cl[c % 2]
                hhc, hlc = hh[c % 2], hl[c % 2]
                npair = KI // 2
                ps = hps.tile([P, 4, CM], F32, tag="h", name=f"h{c}_{blk}")
                h4 = slice(blk * 4, blk * 4 + 4)
                # seed the psum with 32*b1 (broadcast along columns); the
                # matmuls then accumulate with start=False onto it
                nc.scalar.activation(
                    ps[:, :, :C],
                    b1_sb[:, h4, None].broadcast_to([P, 4, C]),
                    AF.Copy)
                for kp in range(npair):
                    ks = slice(2 * kp, 2 * kp + 2)
                    for j in range(4):
                        ht = blk * 4 + j
                        hs = slice(ht * P, (ht + 1) * P)
                        nc.tensor.matmul(
                            ps[:, j, :C], w1h_sb[:, ks, hs], chc[:, ks, :C],
                            start=False, stop=False, perf_mode=DR,
                            skip_group_check=True)
                for kp in range(npair):
                    ks = slice(2 * kp, 2 * kp + 2)
                    for j in range(4):
                        ht = blk * 4 + j
                        hs = slice(ht * P, (ht + 1) * P)
                        nc.tensor.matmul(
                            ps[:, j, :C], w1l_sb[:, ks, hs], chc[:, ks, :C],
                            start=False, stop=False, perf_mode=DR,
                            skip_group_check=True)
                        nc.tensor.matmul(
                            ps[:, j, :C], w1h_sb[:, ks, hs], clc[:, ks, :C],
                            start=False, stop=False,
                            perf_mode=DR, skip_group_check=True)
                tmp = p2t.tile([P, 4, CM], BF16, tag="tmp",
                               name=f"tmp{c}_{blk}")
                nc.scalar.activation(tmp[:, :, :C], ps[:, :, :C],
                                     AF.Relu, scale=ISW)
                nc.gpsimd.tensor_copy(hhc[:, h4, :C], tmp[:, :, :C])
                nc.vector.tensor_sub(hlc[:, h4, :C], tmp[:, :, :C],
                                     hhc[:, h4, :C])

            def mm2(c, tail=False):
                C = CS[c]
                r0 = sum(CS[:c])
                hhc, hlc = hh[c % 2], hl[c % 2]
                npair = KH // 2
                if tail:
                    # drain-friendly: per-oc sequential sweeps so the stt +
                    # out DMA of group N overlap group N+1's matmuls
                    for ss in range(C // P):
                        sx = slice(ss * P, (ss + 1) * P)
                        for oc in range(O // OC):
                            ocs = slice(oc * OC, (oc + 1) * OC)
                            ps = ops.tile([P, OC], F32, tag=f"o{oc}",
                                          name=f"ox{c}_{ss}_{oc}")
                            for kp in range(npair):
                                ks = slice(2 * kp, 2 * kp + 2)
                                nc.tensor.matmul(
                                    ps[:], hhc[:, ks, sx], w2h_sb[:, ks, ocs],
                                    start=(kp == 0), stop=False, perf_mode=DR)
                                nc.tensor.matmul(
                                    ps[:], hlc[:, ks, sx], w2h_sb[:, ks, ocs],
                                    start=False, stop=False, perf_mode=DR)
                                nc.tensor.matmul(
                                    ps[:], hhc[:, ks, sx], w2l_sb[:, ks, ocs],
                                    start=False, stop=(kp == npair - 1),
                                    perf_mode=DR)
                            ot = outp.tile([P, OC], BF16, tag="otx",
                                           name=f"otx{c}_{ss}_{oc}")
                            nc.vector.scalar_tensor_tensor(
                                ot[:], ps[:], ISW, b2full[:, ocs],
                                op0=ALU.mult, op1=ALU.add)
                            nc.sync.dma_start(
                                out[r0 + ss * P:r0 + (ss + 1) * P, ocs], ot[:])
                    return
                for ss in range(C // P):
                    sx = slice(ss * P, (ss + 1) * P)
                    pss = [ops.tile([P, OC], F32, tag=f"o{oc}",
                                    name=f"o{c}_{ss}_{oc}")
                           for oc in range(O // OC)]
                    for kp in range(npair):
                        ks = slice(2 * kp, 2 * kp + 2)
                        for oc in range(O // OC):
                            ps = pss[oc]
                            ocs = slice(oc * OC, (oc + 1) * OC)
                            nc.tensor.matmul(
                                ps[:], hhc[:, ks, sx], w2h_sb[:, ks, ocs],
                                start=(kp == 0), stop=False, perf_mode=DR)
                            nc.tensor.matmul(
                                ps[:], hlc[:, ks, sx], w2h_sb[:, ks, ocs],
                                start=False, stop=False, perf_mode=DR)
                    for kp in range(npair):
                        ks = slice(2 * kp, 2 * kp + 2)
                        for oc in range(O // OC):
                            ps = pss[oc]
                            ocs = slice(oc * OC, (oc + 1) * OC)
                            nc.tensor.matmul(
                                ps[:], hhc[:, ks, sx], w2l_sb[:, ks, ocs],
                                start=False, stop=(kp == npair - 1),
                                perf_mode=DR)
                    ot = outp.tile([P, O], BF16, tag="ot",
                                   name=f"ot{c}_{ss}")
                    for oc in range(O // OC):
                        ocs = slice(oc * OC, (oc + 1) * OC)
                        nc.vector.scalar_tensor_tensor(
                            ot[:, ocs], pss[oc][:], ISW, b2full[:, ocs],
                            op0=ALU.mult, op1=ALU.add)
                    nc.sync.dma_start(
                        out[r0 + ss * P:r0 + (ss + 1) * P, :], ot[:])

            # ---- fused pipeline ----
            nc.sync.dma_start(wg_sb[:, 0:4, :], vwg[:, 0:4, :])
            xs = {0: dma_x(0, split=True)}
            nc.sync.dma_start(wg_sb[:, 4:8, :], vwg[:, 4:8, :])
            xs[1] = dma_x(1, split=True)
            nc.sync.dma_start(bg_sb[:], bgT)
            nc.sync.dma_start(b2_sb[:], b2r)
            nc.gpsimd.partition_broadcast(b2full[:], b2_sb[:])

            decs = {0: p1d.tile([P, KI, CM], BF16, tag="dec", name="dec0")}
            g0 = gate_mm_half(0, xs[0], 0)
            sig_half(0, decs[0], g0, 0)
            g1 = gate_mm_half(0, xs[0], 1)
            sig_half(0, decs[0], g1, 1)
            nc.sync.dma_start(w1h_sb[:, 0:2, :], vw1h[:, 0:2, :])
            dma_x_lo(0, xs[0])
            dma_x_lo(1, xs[1])
            binit = chain(0, decs[0], xs[0], None)
            nc.sync.dma_start(b1_sb[:], b1T)
            for kp in range(1, 4):
                nc.sync.dma_start(w1h_sb[:, 2 * kp:2 * kp + 2, :],
                                  vw1h[:, 2 * kp:2 * kp + 2, :])
            for kp in range(4):
                nc.sync.dma_start(w1l_sb[:, 2 * kp:2 * kp + 2, :],
                                  vw1l[:, 2 * kp:2 * kp + 2, :])

            ghalves = {}
            for c in range(nch):
                nxt = c + 1 < nch
                early = c < 2   # ACT is idle during the DMA-paced start
                if nxt:
                    decs[c + 1] = p1d.tile([P, KI, CM], BF16, tag="dec",
                                           name=f"dec{c + 1}")
                    ghalves[0] = gate_mm_half(c + 1, xs[c + 1], 0)
                    if early:
                        sig_half(c + 1, decs[c + 1], ghalves[0], 0)
                        ghalves[1] = gate_mm_half(c + 1, xs[c + 1], 1)
                        sig_half(c + 1, decs[c + 1], ghalves[1], 1)
                for blk in range(KH // 4):
                    mm1_blk(c, blk)
                    if nxt and not early and blk < 4:
                        sig_one(c + 1, decs[c + 1], ghalves[0], 0, blk)
                    if nxt and not early and blk == 3:
                        # all half-0 sigmoids are emitted: the gps psum
                        # tiles may now be reallocated for half 1
                        ghalves[1] = gate_mm_half(c + 1, xs[c + 1], 1)
                    if nxt and not early and blk >= 4:
                        sig_one(c + 1, decs[c + 1], ghalves[1], 1, blk - 4)
                if c + 2 < nch:
                    xs[c + 2] = dma_x(c + 2)
                if c == 0:
                    dma_w2(0, 8)
                    if nch > 2:
                        dma_x_lo(2, xs[2])
                if c == 1:
                    dma_w2(8, 16)
                if nxt:
                    binit = chain(c + 1, decs.pop(c + 1), xs[c + 1], binit)
                    xs.pop(c)
                if c > 0:
                    mm2(c - 1)
            mm2(nch - 1, tail=True)
    return nc


def make_nc(S_: int = S) -> bass.Bass:
    nc = bacc.Bacc("TRN2", target_bir_lowering=False, debug=False,
                   enable_asserts=False, dynamic_dma_scratch_size=1024)
    build(nc, S_)
    nc.compile()
    return nc


def split8(a: np.ndarray):
    hi = a.astype(NP_E4)
    lo = (a - hi.astype(np.float32)).astype(NP_E4)
    return hi, lo


def prep_in_maps(inputs: dict) -> list[dict]:
    x = np.asarray(inputs["x"], np.float32)
    Wg = np.asarray(inputs["Wg"], np.float32)
    W1 = np.asarray(inputs["W1"], np.float32)
    W2 = np.asarray(inputs["W2"], np.float32)
    bg = np.asarray(inputs["bg"], np.float32)
    b1 = np.asarray(inputs["b1"], np.float32)
    b2 = np.asarray(inputs["b2"], np.float32)

    w1h, w1l = split8(np.ascontiguousarray(W1.T) * SW)   # [j, h]
    w2h, w2l = split8(np.ascontiguousarray(W2.T) * SW)   # [h, o]
    shared = {
        "WghT": (np.ascontiguousarray(Wg.T) * SW).astype(NP_E4),  # [j, i]
        "W1hT": w1h, "W1lT": w1l,
        "W2hT": w2h, "W2lT": w2l,
        "bgT": np.ascontiguousarray(bg.reshape(KI, P).T),  # [p, it]
        "b1T": np.ascontiguousarray((b1 * SW).reshape(KH, P).T),
        "b2r": b2.astype(NP_E4).reshape(1, O),
    }
    S_ = x.shape[1]
    CS = chunk_schedule(S_)
    in_maps = []
    for b in range(B):
        m = dict(shared)
        xT = np.ascontiguousarray(x[b].T)                  # [i, s]
        # per-chunk contiguous layout [P, KI*C per chunk] so each chunk is
        # one DMA with KI*C contiguous bytes per partition row
        blocks = []
        o0 = 0
        for C in CS:
            blk = xT[:, o0:o0 + C].reshape(KI, P, C).transpose(1, 0, 2)
            bh, bl = split8(blk.reshape(P, KI * C))
            blocks.append(bh)
            blocks.append(bl)
            o0 += C
        m["xC"] = np.ascontiguousarray(np.concatenate(blocks, axis=1))
        in_maps.append(m)
    return in_maps


LAST_RESULTS = None


def kernel(**inputs) -> np.ndarray:
    global LAST_RESULTS
    nc = make_nc()
    in_maps = prep_in_maps(inputs)
    res = bass_utils.run_bass_kernel_spmd(nc, in_maps, core_ids=list(range(B)))
    LAST_RESULTS = res
    out = np.stack([r["out"] for r in res.results], axis=0)
    return out.astype(np.float32)
